# revision 10
# baseline (speedup 1.0000x reference)
"""VQ codebook kernel for Trainium2 (8 NeuronCores, SPMD over the head dim).

Per head h (one NeuronCore):
  flat = x[h] reshaped [8192, 512]          (tokens, dim)
  dist[t, c]  = -sqrt(x2[t] + y2[c] - 2*flat@embed[h].T)
  ind[t]      = argmax_c dist[t, c]  (== argmin of the sqrt argument)
  quantize[t] = embed[h][ind[t]]

Implementation per 128-token tile:
  - PE transposes x tile to [d, t] layout, scalar engine writes -2*xT to SBUF
  - PE matmul accumulates psum = (-2x)^T@e^T chunks (K=512 over 4 chunks)
  - DVE tensor_tensor: un = (-y2b) - psum  (= 2xy - y2, argmax(un) = argmin dist)
  - DVE max + max_index on un -> argmin index
  - ACT: v = sqrt(-un + x2_bias); dist = -v
  - GpSimd indirect DMA gathers embed rows by index -> quantize
y2b ([128, 2048] broadcast of per-code norms) is built once per core with a
ones-matmul partition reduction over e^T squared.
"""

import os
import sys

sys.path.insert(0, "/opt/trn_rl_repo")

import numpy as np

H, B, N, D, C = 8, 4, 2048, 512, 2048
BN = B * N          # 8192 tokens per head
P = 128             # partitions
NT = BN // P        # 64 token tiles
KC = D // P         # 4 contraction chunks
NC_ = C // P        # 16 code tiles

MODE = os.environ.get("VQ_MM_MODE", "fp32")  # fp32 | split3

_BUILT = None


def _build():
    import concourse.bass as bass
    import concourse.mybir as mybir
    import concourse.tile as tile
    from concourse import bacc
    from concourse.masks import make_identity

    f32 = mybir.dt.float32
    bf16 = mybir.dt.bfloat16
    u32 = mybir.dt.uint32
    AF = mybir.ActivationFunctionType
    OP = mybir.AluOpType

    nc = bacc.Bacc("TRN2", target_bir_lowering=False, debug=False, num_devices=H)

    x_d = nc.dram_tensor("x", [BN, D], f32, kind="ExternalInput")
    e_d = nc.dram_tensor("embed", [C, D], f32, kind="ExternalInput")
    dist_d = nc.dram_tensor("dist", [BN, C], f32, kind="ExternalOutput")
    quant_d = nc.dram_tensor("quantize", [BN, D], f32, kind="ExternalOutput")
    ind_d = nc.dram_tensor("ind", [P, NT], mybir.dt.int32, kind="ExternalOutput")

    with tile.TileContext(nc) as tc:
        with tc.tile_pool(name="const", bufs=1) as const_p, \
             tc.tile_pool(name="prep", bufs=1) as prep_p, \
             tc.tile_pool(name="xin", bufs=3) as xin_p, \
             tc.tile_pool(name="xt", bufs=3) as xt_p, \
             tc.tile_pool(name="xsq", bufs=2) as xsq_p, \
             tc.tile_pool(name="small", bufs=4) as small_p, \
             tc.tile_pool(name="u", bufs=2) as u_p, \
             tc.tile_pool(name="v", bufs=2) as v_p, \
             tc.tile_pool(name="dout", bufs=2) as dout_p, \
             tc.tile_pool(name="q", bufs=3) as q_p, \
             tc.tile_pool(name="psu", bufs=3, space="PSUM") as psu_p, \
             tc.tile_pool(name="pstr", bufs=2, space="PSUM") as pstr_p:

            ident = const_p.tile([P, P], f32)
            make_identity(nc, ident[:])
            ident_bf = const_p.tile([P, P], bf16)
            make_identity(nc, ident_bf[:])
            ones = const_p.tile([P, P], f32)
            nc.vector.memset(ones[:], 1.0)

            # ---- embed prep: e^T, y2 broadcast, (optional hi/lo split) ----
            prep_head = []                             # deferred x-tile prep hooks
            e_sb = prep_p.tile([P, NC_, D], f32, tag="eprep", name="e_sb")
            for i in range(NC_):
                nc.sync.dma_start(e_sb[:, i, :], e_d.ap()[P * i:P * (i + 1), :])

            # [dp, k, c] = embed[c, 128k+dp]; dead after prep in split3 mode
            if MODE == "split3":
                eT = prep_p.tile([P, KC, C], f32, tag="eT", name="eT")
            else:
                eT = const_p.tile([P, KC, C], f32, tag="eT", name="eT")
            for i in range(NC_):
                tr = pstr_p.tile([P, D], f32, tag="tr", name=f"tr_{i}")
                for k in range(KC):
                    nc.tensor.transpose(
                        tr[:, P * k:P * (k + 1)], e_sb[:, i, P * k:P * (k + 1)],
                        ident[:],
                    )
                nc.scalar.copy(
                    eT[:, :, P * i:P * (i + 1)],
                    tr[:].rearrange("p (k q) -> p k q", k=KC),
                )

            e2 = prep_p.tile([P, KC, C], f32, tag="eprep", name="e2")
            nc.vector.tensor_tensor(out=e2[:], in0=eT[:], in1=eT[:], op=OP.mult)

            y2n = const_p.tile([P, C], f32)            # -y2 broadcast across partitions
            for n in range(KC):
                ps = pstr_p.tile([P, 512], f32, tag="tr", name=f"psy_{n}")
                for k in range(KC):
                    nc.tensor.matmul(
                        ps[:], ones[:], e2[:, k, 512 * n:512 * (n + 1)],
                        start=(k == 0), stop=(k == KC - 1),
                    )
                nc.scalar.mul(y2n[:, 512 * n:512 * (n + 1)], ps[:], -1.0)

            if MODE == "split3":
                ehi = const_p.tile([P, KC, C], bf16)
                elo = const_p.tile([P, KC, C], bf16)
                nc.vector.tensor_copy(ehi[:], eT[:])
                nc.vector.scalar_tensor_tensor(
                    out=elo[:], in0=eT[:], scalar=1.0, in1=ehi[:],
                    op0=OP.mult, op1=OP.subtract,
                )

            # ---- main loop over 64 token tiles (software-pipelined) ----
            ind_all = const_p.tile([P, NT], u32)
            nt_run = int(os.environ.get("VQ_NT", NT))
            state = {}

            def stage_prep(j):
                xj = xin_p.tile([P, D], f32, tag="xj", name=f"xj_{j}")
                nc.sync.dma_start(xj[:], x_d.ap()[P * j:P * (j + 1), :])

                xsq = xsq_p.tile([P, D], f32, tag="xsq", name=f"xsq_{j}")
                x2c = small_p.tile([P, 1], f32, tag="x2c", name=f"x2c_{j}")
                nc.scalar.activation(xsq[:], xj[:], AF.Square, accum_out=x2c[:])

                if MODE == "split3":
                    # split -2x into bf16 hi+lo in natural layout (SBUF ops only)
                    xhn = xsq_p.tile([P, D], bf16, tag="xhn", name=f"xhn_{j}")
                    xln = xsq_p.tile([P, D], bf16, tag="xln", name=f"xln_{j}")
                    nc.scalar.mul(xhn[:], xj[:], -2.0)
                    nc.vector.scalar_tensor_tensor(
                        out=xln[:], in0=xj[:], scalar=-2.0, in1=xhn[:],
                        op0=OP.mult, op1=OP.subtract,
                    )
                    # bf16 PE transposes into one psum tile (hi cols 0:512, lo 512:1024)
                    trhl = pstr_p.tile([P, 2 * D], bf16, tag="tr", name=f"trhl_{j}")
                    for k in range(KC):
                        nc.tensor.transpose(
                            trhl[:, P * k:P * (k + 1)],
                            xhn[:, P * k:P * (k + 1)], ident_bf[:],
                        )
                        nc.tensor.transpose(
                            trhl[:, D + P * k:D + P * (k + 1)],
                            xln[:, P * k:P * (k + 1)], ident_bf[:],
                        )
                    # drain psum on the scalar engine (keeps Vector FIFO free)
                    xhi = xt_p.tile([P, D], bf16, tag="xhi", name=f"xhi_{j}")
                    xlo = xt_p.tile([P, D], bf16, tag="xlo", name=f"xlo_{j}")
                    nc.scalar.copy(xhi[:], trhl[:, 0:D])
                    nc.scalar.copy(xlo[:], trhl[:, D:2 * D])
                    state[j] = (xhi, xlo, x2c)
                else:
                    trx = pstr_p.tile([P, D], f32, tag="tr", name=f"trx_{j}")
                    for k in range(KC):
                        nc.tensor.transpose(
                            trx[:, P * k:P * (k + 1)], xj[:, P * k:P * (k + 1)],
                            ident[:],
                        )
                    xt = xt_p.tile([P, D], f32, tag="xt", name=f"xt_{j}")
                    nc.scalar.mul(xt[:], trx[:], -2.0)
                    state[j] = (xt, None, x2c)

            def stage_main(j):
                lhs_a, lhs_b, x2c = state.pop(j)
                uj = u_p.tile([P, C], f32, tag="uj", name=f"uj_{j}")
                for h in range(2):
                    ps = psu_p.tile([P, 1024], f32, tag="ps", name=f"ps_{j}_{h}")
                    for k in range(KC):
                        for n in range(2):
                            sl = slice(512 * n, 512 * (n + 1))
                            co = slice(1024 * h + 512 * n, 1024 * h + 512 * (n + 1))
                            if MODE == "split3":
                                nc.tensor.matmul(
                                    ps[:, sl], lhs_a[:, P * k:P * (k + 1)],
                                    ehi[:, k, co], start=(k == 0), stop=False,
                                )
                                nc.tensor.matmul(
                                    ps[:, sl], lhs_a[:, P * k:P * (k + 1)],
                                    elo[:, k, co], start=False, stop=False,
                                )
                                nc.tensor.matmul(
                                    ps[:, sl], lhs_b[:, P * k:P * (k + 1)],
                                    ehi[:, k, co], start=False,
                                    stop=(k == KC - 1),
                                )
                            else:
                                nc.tensor.matmul(
                                    ps[:, sl], lhs_a[:, P * k:P * (k + 1)],
                                    eT[:, k, co], start=(k == 0),
                                    stop=(k == KC - 1),
                                )
                    nc.vector.tensor_tensor(
                        out=uj[:, 1024 * h:1024 * (h + 1)],
                        in0=y2n[:, 1024 * h:1024 * (h + 1)], in1=ps[:],
                        op=OP.subtract,
                    )

                mv8 = small_p.tile([P, 8], f32, tag="mv8", name=f"mv8_{j}")
                nc.vector.max(mv8[:], uj[:])
                idx8 = small_p.tile([P, 8], u32, tag="idx8", name=f"idx8_{j}")
                nc.vector.max_index(idx8[:], mv8[:], uj[:])
                nc.vector.tensor_copy(ind_all[:, j:j + 1], idx8[:, 0:1])

                if not os.environ.get("VQ_NO_DIST"):
                    vj = v_p.tile([P, C], f32, tag="vj", name=f"vj_{j}")
                    nc.scalar.activation(vj[:], uj[:], AF.Sqrt, bias=x2c[:], scale=-1.0)
                    dj = dout_p.tile([P, C], f32, tag="dj", name=f"dj_{j}")
                    nc.scalar.mul(dj[:], vj[:], -1.0)
                    nc.scalar.dma_start(dist_d.ap()[P * j:P * (j + 1), :], dj[:])

                qj = q_p.tile([P, D], f32, tag="qj", name=f"qj_{j}")
                if os.environ.get("VQ_NO_GATHER"):
                    nc.vector.memset(qj[:], 0.0)
                else:
                    nc.gpsimd.indirect_dma_start(
                        out=qj[:],
                        out_offset=None,
                        in_=e_d.ap()[:, :],
                        in_offset=bass.IndirectOffsetOnAxis(ap=idx8[:, 0:1], axis=0),
                    )
                nc.scalar.dma_start(quant_d.ap()[P * j:P * (j + 1), :], qj[:])

            stage_prep(0)
            if nt_run > 1:
                stage_prep(1)
            for j in range(nt_run):
                if j + 2 < nt_run:
                    stage_prep(j + 2)
                stage_main(j)

            nc.sync.dma_start(ind_d.ap()[:, :], ind_all[:].bitcast(mybir.dt.int32))

    nc.compile()
    return nc


def _get_built():
    global _BUILT
    if _BUILT is None:
        _BUILT = _build()
    return _BUILT


def kernel(x, embed):
    from concourse.bass_utils import run_bass_kernel_spmd

    x = np.ascontiguousarray(np.asarray(x, dtype=np.float32))
    embed = np.ascontiguousarray(np.asarray(embed, dtype=np.float32))
    nc = _get_built()

    in_maps = [
        {"x": x[h].reshape(BN, D), "embed": embed[h]} for h in range(H)
    ]
    trace = bool(int(os.environ.get("VQ_TRACE", "0")))
    res = run_bass_kernel_spmd(nc, in_maps, core_ids=list(range(H)), trace=trace)
    if trace:
        kernel.last_exec_time_ns = res.exec_time_ns

    quantize = np.stack([res.results[h]["quantize"].reshape(B, N, D) for h in range(H)])
    embed_ind = np.stack(
        [res.results[h]["ind"].T.reshape(B, N) for h in range(H)]
    ).astype(np.int32)
    dist = np.stack([res.results[h]["dist"].reshape(B, N, C) for h in range(H)])
    return quantize, embed_ind, dist


kernel.last_exec_time_ns = None


# revision 11
# speedup vs baseline: 1.1749x; 1.1749x over previous
"""VQ codebook kernel for Trainium2 (8 NeuronCores, SPMD over the head dim).

Per head h (one NeuronCore):
  flat = x[h] reshaped [8192, 512]          (tokens, dim)
  dist[t, c]  = -sqrt(x2[t] + y2[c] - 2*flat@embed[h].T)
  ind[t]      = argmax_c dist[t, c]  (== argmin of the sqrt argument)
  quantize[t] = embed[h][ind[t]]

Implementation per 128-token tile:
  - PE transposes x tile to [d, t] layout, scalar engine writes -2*xT to SBUF
  - PE matmul accumulates psum = (-2x)^T@e^T chunks (K=512 over 4 chunks)
  - DVE tensor_tensor: un = (-y2b) - psum  (= 2xy - y2, argmax(un) = argmin dist)
  - DVE max + max_index on un -> argmin index
  - ACT: v = sqrt(-un + x2_bias); dist = -v
  - GpSimd indirect DMA gathers embed rows by index -> quantize
y2b ([128, 2048] broadcast of per-code norms) is built once per core with a
ones-matmul partition reduction over e^T squared.
"""

import os
import sys

sys.path.insert(0, "/opt/trn_rl_repo")

import numpy as np

H, B, N, D, C = 8, 4, 2048, 512, 2048
BN = B * N          # 8192 tokens per head
P = 128             # partitions
NT = BN // P        # 64 token tiles
KC = D // P         # 4 contraction chunks
NC_ = C // P        # 16 code tiles

MODE = os.environ.get("VQ_MM_MODE", "fp32")  # fp32 | split3

_BUILT = None


def _build():
    import concourse.bass as bass
    import concourse.mybir as mybir
    import concourse.tile as tile
    from concourse import bacc
    from concourse.masks import make_identity

    f32 = mybir.dt.float32
    bf16 = mybir.dt.bfloat16
    u32 = mybir.dt.uint32
    AF = mybir.ActivationFunctionType
    OP = mybir.AluOpType

    nc = bacc.Bacc("TRN2", target_bir_lowering=False, debug=False, num_devices=H)

    x_d = nc.dram_tensor("x", [BN, D], f32, kind="ExternalInput")
    e_d = nc.dram_tensor("embed", [C, D], f32, kind="ExternalInput")
    dist_d = nc.dram_tensor("dist", [BN, C], f32, kind="ExternalOutput")
    quant_d = nc.dram_tensor("quantize", [BN, D], f32, kind="ExternalOutput")
    ind_d = nc.dram_tensor("ind", [P, NT], mybir.dt.int32, kind="ExternalOutput")

    with tile.TileContext(nc) as tc:
        with tc.tile_pool(name="const", bufs=1) as const_p, \
             tc.tile_pool(name="prep", bufs=1) as prep_p, \
             tc.tile_pool(name="xin", bufs=3) as xin_p, \
             tc.tile_pool(name="xt", bufs=3) as xt_p, \
             tc.tile_pool(name="xsq", bufs=2) as xsq_p, \
             tc.tile_pool(name="small", bufs=4) as small_p, \
             tc.tile_pool(name="u", bufs=2) as u_p, \
             tc.tile_pool(name="v", bufs=2) as v_p, \
             tc.tile_pool(name="dout", bufs=2) as dout_p, \
             tc.tile_pool(name="q", bufs=3) as q_p, \
             tc.tile_pool(name="psu", bufs=6, space="PSUM") as psu_p, \
             tc.tile_pool(name="pstr", bufs=2, space="PSUM") as pstr_p:

            ident = const_p.tile([P, P], f32)
            make_identity(nc, ident[:])
            ident_bf = const_p.tile([P, P], bf16)
            make_identity(nc, ident_bf[:])
            ones = const_p.tile([P, P], f32)
            nc.vector.memset(ones[:], 1.0)

            # ---- embed prep: e^T, y2 broadcast, (optional hi/lo split) ----
            prep_head = []                             # deferred x-tile prep hooks
            e_sb = prep_p.tile([P, NC_, D], f32, tag="eprep", name="e_sb")
            for i in range(NC_):
                nc.sync.dma_start(e_sb[:, i, :], e_d.ap()[P * i:P * (i + 1), :])

            # [dp, k, c] = embed[c, 128k+dp]; dead after prep in split3 mode
            if MODE == "split3":
                eT = prep_p.tile([P, KC, C], f32, tag="eT", name="eT")
            else:
                eT = const_p.tile([P, KC, C], f32, tag="eT", name="eT")
            for i in range(NC_):
                tr = pstr_p.tile([P, D], f32, tag="tr", name=f"tr_{i}")
                for k in range(KC):
                    nc.tensor.transpose(
                        tr[:, P * k:P * (k + 1)], e_sb[:, i, P * k:P * (k + 1)],
                        ident[:],
                    )
                nc.scalar.copy(
                    eT[:, :, P * i:P * (i + 1)],
                    tr[:].rearrange("p (k q) -> p k q", k=KC),
                )

            e2 = prep_p.tile([P, KC, C], f32, tag="eprep", name="e2")
            y2n = const_p.tile([P, C], f32)            # -y2 broadcast across partitions
            psy = [
                psu_p.tile([P, 512], f32, tag="ps", name=f"psy_{n}")
                for n in range(KC)
            ]
            for k in range(KC):
                nc.vector.tensor_tensor(
                    out=e2[:, k, :], in0=eT[:, k, :], in1=eT[:, k, :], op=OP.mult,
                )
                for n in range(KC):
                    nc.tensor.matmul(
                        psy[n][:], ones[:], e2[:, k, 512 * n:512 * (n + 1)],
                        start=(k == 0), stop=(k == KC - 1),
                    )
            for n in range(KC):
                nc.scalar.mul(y2n[:, 512 * n:512 * (n + 1)], psy[n][:], -1.0)

            if MODE == "split3":
                ehi = const_p.tile([P, KC, C], bf16)
                elo = const_p.tile([P, KC, C], bf16)
                nc.vector.tensor_copy(ehi[:], eT[:])
                nc.vector.scalar_tensor_tensor(
                    out=elo[:], in0=eT[:], scalar=1.0, in1=ehi[:],
                    op0=OP.mult, op1=OP.subtract,
                )

            # ---- main loop over 64 token tiles (software-pipelined) ----
            ind_all = const_p.tile([P, NT], u32)
            nt_run = int(os.environ.get("VQ_NT", NT))
            state = {}

            def stage_prep(j):
                xj = xin_p.tile([P, D], f32, tag="xj", name=f"xj_{j}")
                nc.sync.dma_start(xj[:], x_d.ap()[P * j:P * (j + 1), :])

                xsq = xsq_p.tile([P, D], f32, tag="xsq", name=f"xsq_{j}")
                x2c = small_p.tile([P, 1], f32, tag="x2c", name=f"x2c_{j}")
                nc.scalar.activation(xsq[:], xj[:], AF.Square, accum_out=x2c[:])

                trx = pstr_p.tile([P, D], f32, tag="tr", name=f"trx_{j}")
                for k in range(KC):
                    nc.tensor.transpose(
                        trx[:, P * k:P * (k + 1)], xj[:, P * k:P * (k + 1)], ident[:],
                    )
                if MODE == "split3":
                    xhi = xt_p.tile([P, D], bf16, tag="xhi", name=f"xhi_{j}")
                    xlo = xt_p.tile([P, D], bf16, tag="xlo", name=f"xlo_{j}")
                    nc.scalar.mul(xhi[:], trx[:], -2.0)
                    nc.vector.scalar_tensor_tensor(
                        out=xlo[:], in0=trx[:], scalar=-2.0, in1=xhi[:],
                        op0=OP.mult, op1=OP.subtract,
                    )
                    state[j] = (xhi, xlo, x2c)
                else:
                    xt = xt_p.tile([P, D], f32, tag="xt", name=f"xt_{j}")
                    nc.scalar.mul(xt[:], trx[:], -2.0)
                    state[j] = (xt, None, x2c)

            def stage_main(j):
                lhs_a, lhs_b, x2c = state.pop(j)
                uj = u_p.tile([P, C], f32, tag="uj", name=f"uj_{j}")
                for h in range(4):
                    ps = psu_p.tile([P, 512], f32, tag="ps", name=f"ps_{j}_{h}")
                    co = slice(512 * h, 512 * (h + 1))
                    for k in range(KC):
                        if MODE == "split3":
                            nc.tensor.matmul(
                                ps[:], lhs_a[:, P * k:P * (k + 1)],
                                ehi[:, k, co], start=(k == 0), stop=False,
                            )
                            nc.tensor.matmul(
                                ps[:], lhs_a[:, P * k:P * (k + 1)],
                                elo[:, k, co], start=False, stop=False,
                            )
                            nc.tensor.matmul(
                                ps[:], lhs_b[:, P * k:P * (k + 1)],
                                ehi[:, k, co], start=False,
                                stop=(k == KC - 1),
                            )
                        else:
                            nc.tensor.matmul(
                                ps[:], lhs_a[:, P * k:P * (k + 1)],
                                eT[:, k, co], start=(k == 0),
                                stop=(k == KC - 1),
                            )
                    nc.vector.tensor_tensor(
                        out=uj[:, co],
                        in0=y2n[:, co], in1=ps[:],
                        op=OP.subtract,
                    )

                mv8 = small_p.tile([P, 8], f32, tag="mv8", name=f"mv8_{j}")
                nc.vector.max(mv8[:], uj[:])
                idx8 = small_p.tile([P, 8], u32, tag="idx8", name=f"idx8_{j}")
                nc.vector.max_index(idx8[:], mv8[:], uj[:])
                nc.vector.tensor_copy(ind_all[:, j:j + 1], idx8[:, 0:1])

                if not os.environ.get("VQ_NO_DIST"):
                    vj = v_p.tile([P, C], f32, tag="vj", name=f"vj_{j}")
                    nc.scalar.activation(vj[:], uj[:], AF.Sqrt, bias=x2c[:], scale=-1.0)
                    dj = dout_p.tile([P, C], f32, tag="dj", name=f"dj_{j}")
                    nc.scalar.mul(dj[:], vj[:], -1.0)
                    nc.scalar.dma_start(dist_d.ap()[P * j:P * (j + 1), :], dj[:])

                qj = q_p.tile([P, D], f32, tag="qj", name=f"qj_{j}")
                if os.environ.get("VQ_NO_GATHER"):
                    nc.vector.memset(qj[:], 0.0)
                else:
                    nc.gpsimd.indirect_dma_start(
                        out=qj[:],
                        out_offset=None,
                        in_=e_d.ap()[:, :],
                        in_offset=bass.IndirectOffsetOnAxis(ap=idx8[:, 0:1], axis=0),
                    )
                nc.scalar.dma_start(quant_d.ap()[P * j:P * (j + 1), :], qj[:])

            stage_prep(0)
            if nt_run > 1:
                stage_prep(1)
            for j in range(nt_run):
                if j + 2 < nt_run:
                    stage_prep(j + 2)
                stage_main(j)

            nc.sync.dma_start(ind_d.ap()[:, :], ind_all[:].bitcast(mybir.dt.int32))

    nc.compile()
    return nc


def _get_built():
    global _BUILT
    if _BUILT is None:
        _BUILT = _build()
    return _BUILT


def kernel(x, embed):
    from concourse.bass_utils import run_bass_kernel_spmd

    x = np.ascontiguousarray(np.asarray(x, dtype=np.float32))
    embed = np.ascontiguousarray(np.asarray(embed, dtype=np.float32))
    nc = _get_built()

    in_maps = [
        {"x": x[h].reshape(BN, D), "embed": embed[h]} for h in range(H)
    ]
    trace = bool(int(os.environ.get("VQ_TRACE", "0")))
    res = run_bass_kernel_spmd(nc, in_maps, core_ids=list(range(H)), trace=trace)
    if trace:
        kernel.last_exec_time_ns = res.exec_time_ns

    quantize = np.stack([res.results[h]["quantize"].reshape(B, N, D) for h in range(H)])
    embed_ind = np.stack(
        [res.results[h]["ind"].T.reshape(B, N) for h in range(H)]
    ).astype(np.int32)
    dist = np.stack([res.results[h]["dist"].reshape(B, N, C) for h in range(H)])
    return quantize, embed_ind, dist


kernel.last_exec_time_ns = None
